# Initial kernel scaffold
#
"""Trainium2 Bass kernel for nn_LogicalGNNLayer (GNN message passing + MLP).

Computation (reference):
    h = term_emb[heads]; t = term_emb[tails]           # gather  [E,B,D]
    agg = segsum(s*(h+pred), tails) + segsum(s*(t+inv), heads)   # [T,B,D]
    agg += EPS*term_emb
    out = relu(agg @ W1 + b1) @ W2 + b2                # [T,B,D]

Strategy:
  - Shard batch B across 8 cores (data parallel, Bc=512 per core); the
    term/edge structure and MLP weights are replicated.
  - The gather/scatter structure depends only on the tiny heads/tails index
    arrays: read them on the host and bake the message structure into the
    kernel as a static program.
  - On-chip layout is transposed: d on partitions, (t, b) on the free axis,
    so the MLP matmuls (which contract D) consume the aggregation output
    directly with no on-device transposes.
  - Aggregation per destination term k (per 128-partition d-tile):
      * the emb slices destined for k are DMA'd as one contiguous tile and
        summed with a wide halving tree (few big DVE ops amortize the
        per-instruction overhead; fp16 tensor_tensor runs in 2x_1p mode),
      * acc[k] = EPS*term[k] + treesum in one scalar_tensor_tensor,
      * + one add per distinct (dst,src) term edge (coefficients merged).
    Units are split between DVE and GpSimd by a cost-model LPT greedy
    (GpSimd tensor ops cost ~3x DVE 2x-mode; GpSimd TENSOR_SCALAR is
    catastrophically slow and is never used).
  - MLP: fp16 matmuls on PE with fp32 PSUM accumulation, software-pipelined
    in chunks of 2 term slots (psum: 2 tags x 2 bufs x 2 banks = 8 banks);
    ReLU and the output epilogue run on the scalar engine out of PSUM.
  - fp16 on-chip and fp16 output DMA (host upcasts) halve HBM traffic; the
    measured rel err vs the fp32 reference is ~5e-4.
"""

import numpy as np

import concourse.bass as bass
import concourse.tile as tile
from concourse import bacc, mybir
from concourse.bass_utils import run_bass_kernel_spmd

T, B, D, H, E = 16, 4096, 256, 512, 32
EPS = 0.1
N_CORES = 8
BC = B // N_CORES            # 512 batch per core
NB = T * BC                  # 8192 free-axis span (t, b)
DT = D // 128                # 2 d-tiles
HT = H // 128                # 4 h-tiles
NMSG = 2 * E                 # 64 directed messages
F16 = mybir.dt.float16
F32 = mybir.dt.float32
F8 = mybir.dt.float8e4

_KERNEL_CACHE = {}


def _messages(heads, tails, signs):
    """Directed message list (dst, src, sign, which_emb, e), sorted by dst."""
    msgs = []
    for e in range(E):
        h, t, s = int(heads[e]), int(tails[e]), float(signs[e])
        assert 0 <= h < T and 0 <= t < T
        msgs.append((t, h, s, 0, e))   # msg_to_tail: acc[t] += s*(term[h]+pred[e])
        msgs.append((h, t, s, 1, e))   # msg_to_head: acc[h] += s*(term[t]+inv[e])
    msgs.sort(key=lambda m: m[0])
    return msgs


def _plan(msgs):
    """Static schedule: emb groups, merged term edges, halving trees,
    DVE/GpSimd assignment per (k, dt) unit."""
    slots = [[] for _ in range(T)]
    for m, (dst, _src, _s, _w, _e) in enumerate(msgs):
        slots[dst].append(m)
    gspan = []
    for k in range(T):
        if slots[k]:
            m0, g = slots[k][0], len(slots[k])
            assert slots[k] == list(range(m0, m0 + g))
            gspan.append((m0, g))
        else:
            gspan.append((0, 0))

    termops = []
    for k in range(T):
        c = {}
        for dst, src, s, _w, _e in msgs:
            if dst == k:
                c[src] = c.get(src, 0.0) + s
        termops.append([("term", src, v)
                        for src, v in sorted(c.items()) if v != 0.0])

    # CSE: a source pair (s1, s2), both coeff 1, shared by >=2 destinations
    # is computed once into a tmp tile; each use replaces 2 adds with 1.
    # Disabled: the tmp builds land on the DVE critical path at their
    # first-use chunk and stall the PE, costing more than they save.
    CSE = False
    tmps = []
    while CSE:
        from collections import Counter
        cnt = Counter()
        for k in range(T):
            ones = sorted(i for _k, i, v in termops[k]
                          if _k == "term" and v == 1.0)
            for a in range(len(ones)):
                for b in range(a + 1, len(ones)):
                    cnt[(ones[a], ones[b])] += 1
        if not cnt:
            break
        (s1, s2), uses = cnt.most_common(1)[0]
        if uses < 2:
            break
        ti = len(tmps)
        tmps.append((s1, s2))
        for k in range(T):
            srcs = {i for _k, i, v in termops[k] if _k == "term" and v == 1.0}
            if s1 in srcs and s2 in srcs:
                termops[k] = [op for op in termops[k]
                              if not (op[0] == "term" and op[1] in (s1, s2)
                                      and op[2] == 1.0)]
                termops[k].append(("tmp", ti, 1.0))

    # halving tree per group: ('fold', i) = slice0 += slice i;
    # ('wide', h) = slices[0:h] += slices[h:2h]
    trees = []
    for k in range(T):
        g = gspan[k][1]
        ops = []
        n = g
        while n > 1:
            if n % 2:
                ops.append(("fold", n - 1))
                n -= 1
            h = n // 2
            ops.append(("wide", h))
            n = h
        trees.append(ops)

    # Engine split: the emb segment-sum runs on PE (identity matmuls into
    # PSUM, immune to DMA/SBUF contention); the DVE evicts psum with a fused
    # EPS*term init (scalar_tensor_tensor, DVE-only, PSUM reads don't
    # contend); term adds balance between DVE and GpSimd using rates
    # measured under DMA contention (DVE TT 780ns, GpSimd TT 1243ns).
    # wide [128,1024] ops covering both d-tiles at once. Clean-rate DVE
    # numbers on purpose: G-heavy assignments balance engine totals but
    # lengthen the per-chunk dependency chains that pace the PE, which
    # measures worse (96.6us vs 88us with measured-contention rates).
    V_TT, G_TT, V_EVICT = 678.0, 2247.0, 1192.0
    units = list(range(T))
    assign = {}
    tv = tg = 0.0
    ucost = {k: len(termops[k]) for k in units}
    for u in sorted(units, key=lambda u: -ucost[u]):
        n = ucost[u]
        m_v = max(tv + V_EVICT + n * V_TT, tg)
        m_g = max(tv + V_EVICT, tg + n * G_TT)
        if n == 0 or m_v <= m_g:
            assign[u] = "v"
            tv += V_EVICT + n * V_TT
        else:
            assign[u] = "g"
            tv += V_EVICT
            tg += n * G_TT
    gmax = max(1, max(g for _m0, g in gspan))
    return gspan, termops, tmps, assign, gmax


def _build(msgs_key, repeats=1, loop=0, bias_zero=(True, True)):
    """Build + compile the per-core SPMD Bass program for a message structure.

    repeats: statically unroll the whole body N times (timing).
    loop: wrap the body in an on-device For_i loop of N iterations (timing).
    bias_zero: (b1 is all-zero, b2 is all-zero) — picks cheaper epilogues.
    """
    key = (msgs_key, repeats, loop, bias_zero)
    if key in _KERNEL_CACHE:
        return _KERNEL_CACHE[key]
    msgs = list(msgs_key)
    AF = mybir.ActivationFunctionType
    OP = mybir.AluOpType
    gspan, termops, tmps, assign, gmax = _plan(msgs)
    b1_zero, b2_zero = bias_zero

    nc = bacc.Bacc("TRN2", target_bir_lowering=False, debug=False,
                   num_devices=N_CORES)
    # termT layout: [p=128, (k, dt, b)] — both d-tiles of a term slot are
    # column-adjacent so evicts/term-adds cover them in one wide op.
    termT = nc.declare_dram_parameter("termT", [128, T * DT * BC], F16,
                                      isOutput=False)
    embT = nc.declare_dram_parameter("embT", [D, NMSG, BC], F8, isOutput=False)
    w1d = nc.declare_dram_parameter("w1", [D, H], F16, isOutput=False)
    w2d = nc.declare_dram_parameter("w2", [H, D], F16, isOutput=False)
    b1d = nc.declare_dram_parameter("b1t", [128, HT], F32, isOutput=False)
    b2d = nc.declare_dram_parameter("b2t", [128, DT], F32, isOutput=False)
    identd = nc.declare_dram_parameter("ident", [128, 2, 128], F8,
                                       isOutput=False)
    outT = nc.declare_dram_parameter("outT", [D, NB], F16, isOutput=True)

    with nc.allow_low_precision(reason="fp16 on-chip aggregation"), \
            tile.TileContext(nc) as tc, \
            tc.tile_pool(name="const", bufs=1) as cpool, \
            tc.tile_pool(name="term", bufs=2) as tpool, \
            tc.tile_pool(name="tmp", bufs=1) as tmppool, \
            tc.tile_pool(name="acc", bufs=1) as apool, \
            tc.tile_pool(name="emb", bufs=10) as epool, \
            tc.tile_pool(name="hid", bufs=3) as hpool, \
            tc.tile_pool(name="out", bufs=6) as opool, \
            tc.tile_pool(name="psagg", bufs=1, space="PSUM") as paggpool, \
            tc.tile_pool(name="psmlp", bufs=2, space="PSUM") as pspool:

        # ---- persistent loads -------------------------------------------
        w1s = []
        w2s = []
        for dt in range(DT):
            w = cpool.tile([128, H], F16, tag=f"w1_{dt}")
            nc.sync.dma_start(w[:], w1d[dt * 128:(dt + 1) * 128, :])
            w1s.append(w)
        for ht in range(HT):
            w = cpool.tile([128, D], F16, tag=f"w2_{ht}")
            nc.sync.dma_start(w[:], w2d[ht * 128:(ht + 1) * 128, :])
            w2s.append(w)
        b1s = cpool.tile([128, HT], F32, tag="b1")
        nc.sync.dma_start(b1s[:], b1d[:])
        b2s = cpool.tile([128, DT], F32, tag="b2")
        nc.sync.dma_start(b2s[:], b2d[:])
        ident = cpool.tile([128, 2, 128], F8, tag="ident")
        nc.sync.dma_start(ident[:], identd[:])

        carry = {"hids": None}

        def body(final=True):
            terms = [None]
            accs = [None] * T
            paggs = {}
            hids = {}

            def emit_embmm(c):
                # Segment-sum of the (fp8) emb slices for term slots 2c, 2c+1
                # as identity matmuls accumulating in PSUM: PE is immune to
                # the DMA/SBUF contention that triples DVE/GpSimd op cost,
                # and fp8 DoubleRow sums two slices per 512-row pass.
                DR = mybir.MatmulPerfMode.DoubleRow
                for ki in range(2):
                    k = 2 * c + ki
                    m0, g = gspan[k]
                    if not g:
                        continue
                    pagg = paggpool.tile([128, 1024], F32, tag=f"pagg_{ki}")
                    paggs[k] = pagg
                    for dt in range(DT):
                        et = epool.tile([128, gmax, BC], F8, tag="emb")
                        nc.sync.dma_start(
                            et[:, :g, :],
                            embT[dt * 128:(dt + 1) * 128, m0:m0 + g, :])
                        half = pagg[:, dt * 512:(dt + 1) * 512]
                        for j in range(0, g - 1, 2):
                            nc.tensor.matmul(
                                half, ident[:], et[:, j:j + 2, :],
                                perf_mode=DR,
                                start=(j == 0), stop=(j + 2 == g))
                        if g % 2:
                            nc.tensor.matmul(
                                half, ident[:, 0, :], et[:, g - 1:g, :],
                                start=(g == 1), stop=True)

            def emit_evict(k):
                # evict: acc = EPS*term[k] + psum segsum, one wide op over
                # both d-tiles (DVE only — TensorScalarPtr is unsupported on
                # Pool, and PSUM reads don't contend with DMA SBUF writes).
                g = gspan[k][1]
                acc = apool.tile([128, 2 * BC], F16, tag=f"acc_{k}")
                accs[k] = acc
                tk = terms[0][:, k * 2 * BC:(k + 1) * 2 * BC]
                if g:
                    nc.vector.scalar_tensor_tensor(
                        acc[:], tk, EPS, paggs[k][:], OP.mult, OP.add)
                else:
                    nc.vector.tensor_scalar_mul(acc[:], tk, EPS)

            def emit_adds(k):
                # term-edge adds on the assigned engine
                eng = nc.vector if assign[k] == "v" else nc.gpsimd
                acc = accs[k]
                for kind, idx, c in termops[k]:
                    if kind == "tmp":
                        ts = tmptiles[idx][:]
                    else:
                        ts = terms[0][:, idx * 2 * BC:(idx + 1) * 2 * BC]
                    if c == 1.0:
                        eng.tensor_add(acc[:], acc[:], ts)
                    elif c == -1.0:
                        eng.tensor_sub(acc[:], acc[:], ts)
                    else:
                        nc.vector.scalar_tensor_tensor(acc[:], ts, c, acc[:],
                                                       OP.mult, OP.add)

            def emit_l1(c):
                for ht in range(HT):
                    ps = pspool.tile([128, 1024], F32, tag="mlp")
                    for dt in range(DT):
                        w = w1s[dt][:, ht * 128:(ht + 1) * 128]
                        for ki in range(2):
                            k = 2 * c + ki
                            nc.tensor.matmul(
                                ps[:, ki * 512:(ki + 1) * 512], w,
                                accs[k][:, dt * 512:(dt + 1) * 512],
                                start=(dt == 0), stop=(dt == DT - 1))
                    hid = hpool.tile([128, 1024], F16, tag=f"hid_{ht}")
                    if b1_zero:
                        nc.scalar.activation(hid[:], ps[:], AF.Relu,
                                             bias=0.0, scale=1.0)
                    else:
                        nc.scalar.activation(hid[:], ps[:], AF.Relu,
                                             bias=b1s[:, ht:ht + 1], scale=1.0)
                    hids[(c, ht)] = hid

            def emit_l2(c, hidmap=None):
                hidmap = hidmap if hidmap is not None else hids
                for dt2 in range(DT):
                    ps2 = pspool.tile([128, 1024], F32, tag="mlp")
                    for ht in range(HT):
                        w = w2s[ht][:, dt2 * 128:(dt2 + 1) * 128]
                        for ki in range(2):
                            nc.tensor.matmul(
                                ps2[:, ki * 512:(ki + 1) * 512], w,
                                hidmap[(c, ht)][:, ki * 512:(ki + 1) * 512],
                                start=(ht == 0), stop=(ht == HT - 1))
                    ot = opool.tile([128, 1024], F16, tag="ot")
                    if b2_zero:
                        nc.scalar.activation(ot[:], ps2[:], AF.Copy,
                                             bias=0.0, scale=1.0)
                    else:
                        nc.scalar.activation(ot[:], ps2[:], AF.Identity,
                                             bias=b2s[:, dt2:dt2 + 1],
                                             scale=1.0)
                    nc.sync.dma_start(
                        outT[dt2 * 128:(dt2 + 1) * 128,
                             2 * c * BC:(2 * c + 2) * BC], ot[:])

            # emb DMAs for the first two chunks go ahead of the term DMA so
            # the PE gets embmm work at rep start (kills the rep-boundary
            # gap); units only need `terms` a little later.
            emit_embmm(0)
            emit_embmm(1)
            tt = tpool.tile([128, T * DT * BC], F16, tag="term")
            nc.sync.dma_start(tt[:], termT[:, :])
            terms[0] = tt
            if carry["hids"] is not None:
                # previous rep's last-chunk L2, pipelined across the rep
                # seam so the PE isn't blocked in-order on ReLU(7)
                emit_l2(T // 2 - 1, carry["hids"])
                carry["hids"] = None
            tmptiles = {}
            first_use = {}
            for c in range(T // 2):
                for ki in range(2):
                    for kind, idx, _c in termops[2 * c + ki]:
                        if kind == "tmp":
                            first_use.setdefault(idx, c)
            for c in range(T // 2):
                # lazy tmp builds, just before their first consumer chunk
                for ti, (s1, s2) in enumerate(tmps):
                    if first_use.get(ti) == c:
                        tm = tmppool.tile([128, 2 * BC], F16, tag=f"tmp_{ti}")
                        tmptiles[ti] = tm
                        nc.vector.tensor_add(
                            tm[:], terms[0][:, s1 * 2 * BC:(s1 + 1) * 2 * BC],
                            terms[0][:, s2 * 2 * BC:(s2 + 1) * 2 * BC])
                k0, k1 = 2 * c, 2 * c + 1
                if assign[k0] != assign[k1]:
                    # cross-engine chunk: both evicts (DVE) first so the
                    # gpsimd unit's adds start early
                    emit_evict(k0)
                    emit_evict(k1)
                    emit_adds(k0)
                    emit_adds(k1)
                else:
                    emit_evict(k0)
                    emit_adds(k0)
                    emit_evict(k1)
                    emit_adds(k1)
                if c + 2 < T // 2:
                    emit_embmm(c + 2)
                if c > 0:
                    emit_l2(c - 1)
                emit_l1(c)
            if final:
                emit_l2(T // 2 - 1)
            else:
                carry["hids"] = hids

        if loop:
            ET = mybir.EngineType
            with tc.For_i(0, loop, 1,
                          hint_engines=(ET.PE, ET.DVE, ET.Activation, ET.SP)):
                body()
        else:
            for rep in range(repeats):
                body(final=(rep == repeats - 1))

    nc.compile()
    _KERNEL_CACHE[key] = nc
    return nc


def _prep_inputs(term_emb, pred_emb, inv_pred_emb, W1, b1, W2, b2, msgs):
    """Shard/transpose/cast host-side into the per-core device layouts."""
    import ml_dtypes
    f8 = ml_dtypes.float8_e4m3
    t16 = term_emb.astype(np.float16)
    emb = np.empty((NMSG, B, D), f8)
    for m, (_dst, _src, s, which, e) in enumerate(msgs):
        arr = pred_emb if which == 0 else inv_pred_emb
        if s == 1.0:
            emb[m] = arr[e].astype(f8)
        else:
            emb[m] = (s * arr[e]).astype(f8)
    w1_16 = np.ascontiguousarray(W1.astype(np.float16))
    w2_16 = np.ascontiguousarray(W2.astype(np.float16))
    b1t = np.ascontiguousarray(b1.astype(np.float32).reshape(HT, 128).T)
    b2t = np.ascontiguousarray(b2.astype(np.float32).reshape(DT, 128).T)
    ident = np.broadcast_to(np.eye(128, dtype=f8)[:, None, :],
                            (128, 2, 128))
    ident = np.ascontiguousarray(ident)
    in_maps = []
    for c in range(N_CORES):
        sl = slice(c * BC, (c + 1) * BC)
        termTc = np.ascontiguousarray(
            t16[:, sl, :].transpose(2, 0, 1).reshape(DT, 128, T, BC)
            .transpose(1, 2, 0, 3)).reshape(128, T * DT * BC)
        embTc = np.ascontiguousarray(
            emb[:, sl, :].transpose(2, 0, 1)).reshape(D, NMSG, BC)
        in_maps.append(dict(termT=termTc, embT=embTc, w1=w1_16, w2=w2_16,
                            b1t=b1t, b2t=b2t, ident=ident))
    return in_maps


def kernel(term_emb, pred_emb, inv_pred_emb, signs, W1, b1, W2, b2,
           heads, tails):
    term_emb = np.asarray(term_emb, dtype=np.float32)
    pred_emb = np.asarray(pred_emb, dtype=np.float32)
    inv_pred_emb = np.asarray(inv_pred_emb, dtype=np.float32)
    signs = np.asarray(signs, dtype=np.float32)
    W1 = np.asarray(W1, dtype=np.float32)
    b1 = np.asarray(b1, dtype=np.float32)
    W2 = np.asarray(W2, dtype=np.float32)
    b2 = np.asarray(b2, dtype=np.float32)
    heads = np.asarray(heads).astype(np.int64)
    tails = np.asarray(tails).astype(np.int64)

    msgs = _messages(heads, tails, signs)
    bias_zero = (not b1.any(), not b2.any())
    nc = _build(tuple(msgs), bias_zero=bias_zero)
    in_maps = _prep_inputs(term_emb, pred_emb, inv_pred_emb, W1, b1, W2, b2,
                           msgs)
    res = run_bass_kernel_spmd(nc, in_maps, list(range(N_CORES)))

    out = np.empty((T, B, D), np.float32)
    for c in range(N_CORES):
        o = res.results[c]["outT"].astype(np.float32)
        out[:, c * BC:(c + 1) * BC, :] = o.reshape(D, T, BC).transpose(1, 2, 0)
    return out



# revision 4
# speedup vs baseline: 1.0053x; 1.0053x over previous
"""Trainium2 Bass kernel for nn_LogicalGNNLayer (GNN message passing + MLP).

Computation (reference):
    h = term_emb[heads]; t = term_emb[tails]           # gather  [E,B,D]
    agg = segsum(s*(h+pred), tails) + segsum(s*(t+inv), heads)   # [T,B,D]
    agg += EPS*term_emb
    out = relu(agg @ W1 + b1) @ W2 + b2                # [T,B,D]

Strategy:
  - Shard batch B across 8 cores (data parallel, Bc=512 per core); the
    term/edge structure and MLP weights are replicated.
  - The gather/scatter structure depends only on the tiny heads/tails index
    arrays: read them on the host and bake the message structure into the
    kernel as a static program.
  - On-chip layout is transposed: d on partitions, (t, b) on the free axis,
    so the MLP matmuls (which contract D) consume the aggregation output
    directly with no on-device transposes.
  - The PE is the bottleneck engine and is issue-rate-bound at ~1 moving
    column/cycle @2.4GHz (measured: 216ns per 512-col matmul regardless of
    dtype/perf-mode; fp8 DoubleRow only buys K=256-per-column packing, not
    column rate). Per-rep PE floor = 68 segsum passes + 256 MLP passes.
  - Aggregation per destination term k (per 128-partition d-tile):
      * emb message slices for k are DMA'd as one contiguous tile and
        segment-summed on the PE as fp8 DoubleRow identity matmuls into
        PSUM (2 message tiles per 512-col pass),
      * odd-g slots get a host-prepared EPS*term[k] fp8 "pseudo-message"
        tile that rides the otherwise-wasted half of the last DR pass for
        free; their eviction then fuses the first term-add with the PSUM
        read (plain TT) instead of a separate EPS STT,
      * even slots: acc[k] = (EPS+c_self)*term[k] + psum in one
        scalar_tensor_tensor (self-edge coefficients folded in),
      * + one add per remaining distinct (dst,src) term edge.
    Adds are split between DVE and GpSimd by a cost-model LPT greedy.
  - term_emb is loaded as 16 per-slot tiles (not one 4MB block) in
    first-use order, interleaved with the emb-tile prefetch stream, so no
    DVE/PE consumer waits on a monolithic transfer and emb tiles are never
    queued behind term bytes on the DMA engines.
  - MLP: fp16 matmuls on PE with fp32 PSUM accumulation, software-pipelined
    in chunks of 2 term slots (psum: 2 tags x 2 bufs x 2 banks = 8 banks);
    ReLU and the output epilogue run on the scalar engine out of PSUM.
  - fp16 on-chip and fp16 output DMA (host upcasts) halve HBM traffic;
    emb tiles are fp8 (measured rel err vs the fp32 reference ~1.4e-2,
    within the 2e-2 gate; no further precision reduction fits the budget).
"""

import numpy as np

import concourse.bass as bass
import concourse.tile as tile
from concourse import bacc, mybir
from concourse.bass_utils import run_bass_kernel_spmd

T, B, D, H, E = 16, 4096, 256, 512, 32
EPS = 0.1
N_CORES = 8
BC = B // N_CORES            # 512 batch per core
NB = T * BC                  # 8192 free-axis span (t, b)
DT = D // 128                # 2 d-tiles
HT = H // 128                # 4 h-tiles
F16 = mybir.dt.float16
F32 = mybir.dt.float32
F8 = mybir.dt.float8e4

_KERNEL_CACHE = {}


def _messages(heads, tails, signs):
    """Directed message list (dst, src, sign, which_emb, e), sorted by dst.

    which_emb: 0 = pred_emb[e], 1 = inv_pred_emb[e], 2 = EPS*term_emb[e]
    (pseudo-message appended to odd-degree slots so every slot has an even
    message count: the pseudo tile fills the second half of a DoubleRow
    pass for free and replaces the separate EPS eviction op)."""
    msgs = []
    for e in range(E):
        h, t, s = int(heads[e]), int(tails[e]), float(signs[e])
        assert 0 <= h < T and 0 <= t < T
        msgs.append((t, h, s, 0, e))   # msg_to_tail: acc[t] += s*(term[h]+pred[e])
        msgs.append((h, t, s, 1, e))   # msg_to_head: acc[h] += s*(term[t]+inv[e])
    cnt = [0] * T
    for dst, _s, _sg, _w, _e in msgs:
        cnt[dst] += 1
    for k in range(T):
        if cnt[k] % 2:
            msgs.append((k, -1, 1.0, 2, k))
    msgs.sort(key=lambda m: m[0])      # stable: pseudo stays last in slot
    return msgs


def _plan(msgs):
    """Static schedule: emb groups, merged term edges, evict fusion,
    DVE/GpSimd assignment per slot."""
    slots = [[] for _ in range(T)]
    for m, (dst, _src, _s, _w, _e) in enumerate(msgs):
        slots[dst].append(m)
    gspan = []
    for k in range(T):
        if slots[k]:
            m0, g = slots[k][0], len(slots[k])
            assert slots[k] == list(range(m0, m0 + g))
            gspan.append((m0, g))
        else:
            gspan.append((0, 0))
    pseudo = {k for (dst, _src, _s, w, _e) in msgs for k in [dst] if w == 2}

    termops = []
    eps_scalar = [EPS] * T
    for k in range(T):
        c = {}
        for dst, src, s, w, _e in msgs:
            if dst == k and w != 2:
                c[src] = c.get(src, 0.0) + s
        ops = [("term", src, v) for src, v in sorted(c.items()) if v != 0.0]
        if k not in pseudo:
            # fold a self-edge coefficient into the EPS eviction scalar
            keep = []
            for kind, src, v in ops:
                if src == k:
                    eps_scalar[k] += v
                else:
                    keep.append((kind, src, v))
            ops = keep
        termops.append(ops)

    # Engine split: the emb segment-sum runs on PE (identity matmuls into
    # PSUM, immune to DMA/SBUF contention); the DVE evicts psum (fused
    # with EPS*term or the first term-add); remaining term adds balance
    # between DVE and GpSimd. Clean-rate DVE numbers on purpose: G-heavy
    # assignments balance engine totals but lengthen the per-chunk
    # dependency chains that pace the PE, which measures worse.
    V_TT, G_TT, V_EVICT = 678.0, 2247.0, 1192.0
    units = list(range(T))
    assign = {}
    tv = tg = 0.0
    ucost = {}
    for k in units:
        n = len(termops[k])
        if k in pseudo and n:
            n -= 1                      # first add fused into the evict
        ucost[k] = n
    for u in sorted(units, key=lambda u: -ucost[u]):
        n = ucost[u]
        m_v = max(tv + V_EVICT + n * V_TT, tg)
        m_g = max(tv + V_EVICT, tg + n * G_TT)
        if n == 0 or m_v <= m_g:
            assign[u] = "v"
            tv += V_EVICT + n * V_TT
        else:
            assign[u] = "g"
            tv += V_EVICT
            tg += n * G_TT
    gmax = max(1, max(g for _m0, g in gspan))
    return gspan, termops, eps_scalar, pseudo, assign, gmax


def _build(msgs_key, repeats=1, loop=0, bias_zero=(True, True)):
    """Build + compile the per-core SPMD Bass program for a message structure.

    repeats: statically unroll the whole body N times (timing).
    loop: wrap the body in an on-device For_i loop of N iterations (timing).
    bias_zero: (b1 is all-zero, b2 is all-zero) — picks cheaper epilogues.
    """
    key = (msgs_key, repeats, loop, bias_zero)
    if key in _KERNEL_CACHE:
        return _KERNEL_CACHE[key]
    msgs = list(msgs_key)
    NMSG = len(msgs)
    AF = mybir.ActivationFunctionType
    OP = mybir.AluOpType
    gspan, termops, eps_scalar, pseudo, assign, gmax = _plan(msgs)
    b1_zero, b2_zero = bias_zero

    # term-slot DMA issue order: first-use order of each slot across the
    # chunk pipeline (evict dst first, then add sources in emission order)
    torder = []
    for c in range(T // 2):
        for k in (2 * c, 2 * c + 1):
            if k not in torder:
                torder.append(k)
            for _kind, src, _v in termops[k]:
                if src not in torder:
                    torder.append(src)
    assert sorted(torder) == list(range(T))

    nc = bacc.Bacc("TRN2", target_bir_lowering=False, debug=False,
                   num_devices=N_CORES)
    # termT layout: [p=128, (k, dt, b)] — both d-tiles of a term slot are
    # column-adjacent so evicts/term-adds cover them in one wide op.
    termT = nc.declare_dram_parameter("termT", [128, T * DT * BC], F16,
                                      isOutput=False)
    embT = nc.declare_dram_parameter("embT", [D, NMSG, BC], F8, isOutput=False)
    w1d = nc.declare_dram_parameter("w1", [D, H], F16, isOutput=False)
    w2d = nc.declare_dram_parameter("w2", [H, D], F16, isOutput=False)
    b1d = nc.declare_dram_parameter("b1t", [128, HT], F32, isOutput=False)
    b2d = nc.declare_dram_parameter("b2t", [128, DT], F32, isOutput=False)
    identd = nc.declare_dram_parameter("ident", [128, 2, 128], F8,
                                       isOutput=False)
    outT = nc.declare_dram_parameter("outT", [D, NB], F16, isOutput=True)

    with nc.allow_low_precision(reason="fp16 on-chip aggregation"), \
            tile.TileContext(nc) as tc, \
            tc.tile_pool(name="const", bufs=1) as cpool, \
            tc.tile_pool(name="term", bufs=2) as tpool, \
            tc.tile_pool(name="acc", bufs=1) as apool, \
            tc.tile_pool(name="emb", bufs=16) as epool, \
            tc.tile_pool(name="hid", bufs=3) as hpool, \
            tc.tile_pool(name="out", bufs=6) as opool, \
            tc.tile_pool(name="psagg", bufs=1, space="PSUM") as paggpool, \
            tc.tile_pool(name="psmlp", bufs=2, space="PSUM") as pspool:

        # ---- persistent loads -------------------------------------------
        w1s = []
        w2s = []
        for dt in range(DT):
            w = cpool.tile([128, H], F16, tag=f"w1_{dt}")
            nc.sync.dma_start(w[:], w1d[dt * 128:(dt + 1) * 128, :])
            w1s.append(w)
        for ht in range(HT):
            w = cpool.tile([128, D], F16, tag=f"w2_{ht}")
            nc.sync.dma_start(w[:], w2d[ht * 128:(ht + 1) * 128, :])
            w2s.append(w)
        b1s = cpool.tile([128, HT], F32, tag="b1")
        nc.sync.dma_start(b1s[:], b1d[:])
        b2s = cpool.tile([128, DT], F32, tag="b2")
        nc.sync.dma_start(b2s[:], b2d[:])
        ident = cpool.tile([128, 2, 128], F8, tag="ident")
        nc.sync.dma_start(ident[:], identd[:])

        carry = {"hids": None}

        def body(final=True):
            tterm = [None] * T
            accs = [None] * T
            paggs = {}
            hids = {}
            emb_tiles = {}

            def emit_term_dma(k):
                tt = tpool.tile([128, 2 * BC], F16, tag=f"term_{k}")
                nc.sync.dma_start(tt[:], termT[:, k * 2 * BC:(k + 1) * 2 * BC])
                tterm[k] = tt

            def emit_emb_dma(c):
                for ki in range(2):
                    k = 2 * c + ki
                    m0, g = gspan[k]
                    if not g:
                        continue
                    for dt in range(DT):
                        et = epool.tile([128, gmax, BC], F8, tag="emb")
                        nc.sync.dma_start(
                            et[:, :g, :],
                            embT[dt * 128:(dt + 1) * 128, m0:m0 + g, :])
                        emb_tiles[(k, dt)] = et

            def emit_seg_mm(c):
                # Segment-sum of the (fp8) emb slices for term slots 2c, 2c+1
                # as identity matmuls accumulating in PSUM: PE is immune to
                # the DMA/SBUF contention that triples DVE/GpSimd op cost.
                # Message counts are even (pseudo-messages pad odd slots) so
                # every pass is a DoubleRow pair.
                DR = mybir.MatmulPerfMode.DoubleRow
                for ki in range(2):
                    k = 2 * c + ki
                    g = gspan[k][1]
                    if not g:
                        continue
                    pagg = paggpool.tile([128, 1024], F32, tag=f"pagg_{ki}")
                    paggs[k] = pagg
                    for dt in range(DT):
                        et = emb_tiles[(k, dt)]
                        half = pagg[:, dt * 512:(dt + 1) * 512]
                        for j in range(0, g - 1, 2):
                            nc.tensor.matmul(
                                half, ident[:], et[:, j:j + 2, :],
                                perf_mode=DR,
                                start=(j == 0), stop=(j + 2 >= g))
                        if g % 2:
                            nc.tensor.matmul(
                                half, ident[:, 0, :], et[:, g - 1:g, :],
                                start=(g == 1), stop=True)

            def emit_evict(k):
                # evict psum -> sbuf acc in one wide op over both d-tiles
                # (DVE only — TensorScalarPtr is unsupported on Pool, and
                # PSUM reads don't contend with DMA SBUF writes).
                # Even slots: acc = (EPS+c_self)*term[k] + psum.
                # Pseudo slots: the EPS*term[k] already rode the emb stream;
                # fuse the first term-add with the psum read instead.
                g = gspan[k][1]
                acc = apool.tile([128, 2 * BC], F16, tag=f"acc_{k}")
                accs[k] = acc
                if not g:
                    nc.vector.tensor_scalar_mul(acc[:], tterm[k][:],
                                                eps_scalar[k])
                    return
                if k in pseudo and termops[k]:
                    _kind, src, cv = termops[k][0]
                    ts = tterm[src][:]
                    if cv == 1.0:
                        nc.vector.tensor_add(acc[:], paggs[k][:], ts)
                    else:
                        nc.vector.scalar_tensor_tensor(
                            acc[:], ts, cv, paggs[k][:], OP.mult, OP.add)
                elif k in pseudo:
                    nc.vector.tensor_scalar_add(acc[:], paggs[k][:], 0.0)
                else:
                    nc.vector.scalar_tensor_tensor(
                        acc[:], tterm[k][:], eps_scalar[k], paggs[k][:],
                        OP.mult, OP.add)

            def emit_adds(k):
                # term-edge adds on the assigned engine
                eng = nc.vector if assign[k] == "v" else nc.gpsimd
                acc = accs[k]
                ops = termops[k]
                if k in pseudo and ops:
                    ops = ops[1:]       # fused into the evict
                for _kind, idx, c in ops:
                    ts = tterm[idx][:]
                    if c == 1.0:
                        eng.tensor_add(acc[:], acc[:], ts)
                    elif c == -1.0:
                        eng.tensor_sub(acc[:], acc[:], ts)
                    else:
                        nc.vector.scalar_tensor_tensor(acc[:], ts, c, acc[:],
                                                       OP.mult, OP.add)

            def emit_l1(c):
                for ht in range(HT):
                    ps = pspool.tile([128, 1024], F32, tag="mlp")
                    for dt in range(DT):
                        w = w1s[dt][:, ht * 128:(ht + 1) * 128]
                        for ki in range(2):
                            k = 2 * c + ki
                            nc.tensor.matmul(
                                ps[:, ki * 512:(ki + 1) * 512], w,
                                accs[k][:, dt * 512:(dt + 1) * 512],
                                start=(dt == 0), stop=(dt == DT - 1))
                    hid = hpool.tile([128, 1024], F16, tag=f"hid_{ht}")
                    if b1_zero:
                        nc.scalar.activation(hid[:], ps[:], AF.Relu,
                                             bias=0.0, scale=1.0)
                    else:
                        nc.scalar.activation(hid[:], ps[:], AF.Relu,
                                             bias=b1s[:, ht:ht + 1], scale=1.0)
                    hids[(c, ht)] = hid

            def emit_l2(c, hidmap=None):
                hidmap = hidmap if hidmap is not None else hids
                for dt2 in range(DT):
                    ps2 = pspool.tile([128, 1024], F32, tag="mlp")
                    for ht in range(HT):
                        w = w2s[ht][:, dt2 * 128:(dt2 + 1) * 128]
                        for ki in range(2):
                            nc.tensor.matmul(
                                ps2[:, ki * 512:(ki + 1) * 512], w,
                                hidmap[(c, ht)][:, ki * 512:(ki + 1) * 512],
                                start=(ht == 0), stop=(ht == HT - 1))
                    ot = opool.tile([128, 1024], F16, tag="ot")
                    if b2_zero:
                        nc.scalar.activation(ot[:], ps2[:], AF.Copy,
                                             bias=0.0, scale=1.0)
                    else:
                        nc.scalar.activation(ot[:], ps2[:], AF.Identity,
                                             bias=b2s[:, dt2:dt2 + 1],
                                             scale=1.0)
                    nc.sync.dma_start(
                        outT[dt2 * 128:(dt2 + 1) * 128,
                             2 * c * BC:(2 * c + 2) * BC], ot[:])

            # DMA issue order: emb tiles for the first chunks lead (the PE
            # consumes them first), term-slot tiles follow in first-use
            # order interleaved with further emb prefetch so neither stream
            # queues behind the other on the DMA engines. Seg matmuls stay
            # in their pipelined position (2 chunks ahead of eviction).
            emit_emb_dma(0)
            emit_emb_dma(1)
            for k in torder[0:4]:
                emit_term_dma(k)
            emit_emb_dma(2)
            for k in torder[4:10]:
                emit_term_dma(k)
            emit_emb_dma(3)
            for k in torder[10:16]:
                emit_term_dma(k)
            emit_seg_mm(0)
            emit_seg_mm(1)
            if carry["hids"] is not None:
                # previous rep's last-chunk L2, pipelined across the rep
                # seam so the PE isn't blocked in-order on ReLU(7)
                emit_l2(T // 2 - 1, carry["hids"])
                carry["hids"] = None
            for c in range(T // 2):
                k0, k1 = 2 * c, 2 * c + 1
                if assign[k0] != assign[k1]:
                    # cross-engine chunk: both evicts (DVE) first so the
                    # gpsimd unit's adds start early
                    emit_evict(k0)
                    emit_evict(k1)
                    emit_adds(k0)
                    emit_adds(k1)
                else:
                    emit_evict(k0)
                    emit_adds(k0)
                    emit_evict(k1)
                    emit_adds(k1)
                if c + 4 < T // 2:
                    emit_emb_dma(c + 4)
                if c + 2 < T // 2:
                    emit_seg_mm(c + 2)
                if c > 0:
                    emit_l2(c - 1)
                emit_l1(c)
            if final:
                emit_l2(T // 2 - 1)
            else:
                carry["hids"] = hids

        if loop:
            ET = mybir.EngineType
            with tc.For_i(0, loop, 1,
                          hint_engines=(ET.PE, ET.DVE, ET.Activation, ET.SP)):
                body()
        else:
            for rep in range(repeats):
                body(final=(rep == repeats - 1))

    nc.compile()
    _KERNEL_CACHE[key] = nc
    return nc


def _prep_inputs(term_emb, pred_emb, inv_pred_emb, W1, b1, W2, b2, msgs):
    """Shard/transpose/cast host-side into the per-core device layouts."""
    import ml_dtypes
    f8 = ml_dtypes.float8_e4m3
    NMSG = len(msgs)
    t16 = term_emb.astype(np.float16)
    emb = np.empty((NMSG, B, D), f8)
    for m, (_dst, _src, s, which, e) in enumerate(msgs):
        if which == 2:
            emb[m] = (EPS * term_emb[e]).astype(f8)
            continue
        arr = pred_emb if which == 0 else inv_pred_emb
        if s == 1.0:
            emb[m] = arr[e].astype(f8)
        else:
            emb[m] = (s * arr[e]).astype(f8)
    w1_16 = np.ascontiguousarray(W1.astype(np.float16))
    w2_16 = np.ascontiguousarray(W2.astype(np.float16))
    b1t = np.ascontiguousarray(b1.astype(np.float32).reshape(HT, 128).T)
    b2t = np.ascontiguousarray(b2.astype(np.float32).reshape(DT, 128).T)
    ident = np.broadcast_to(np.eye(128, dtype=f8)[:, None, :],
                            (128, 2, 128))
    ident = np.ascontiguousarray(ident)
    in_maps = []
    for c in range(N_CORES):
        sl = slice(c * BC, (c + 1) * BC)
        termTc = np.ascontiguousarray(
            t16[:, sl, :].transpose(2, 0, 1).reshape(DT, 128, T, BC)
            .transpose(1, 2, 0, 3)).reshape(128, T * DT * BC)
        embTc = np.ascontiguousarray(
            emb[:, sl, :].transpose(2, 0, 1)).reshape(D, NMSG, BC)
        in_maps.append(dict(termT=termTc, embT=embTc, w1=w1_16, w2=w2_16,
                            b1t=b1t, b2t=b2t, ident=ident))
    return in_maps


def kernel(term_emb, pred_emb, inv_pred_emb, signs, W1, b1, W2, b2,
           heads, tails):
    term_emb = np.asarray(term_emb, dtype=np.float32)
    pred_emb = np.asarray(pred_emb, dtype=np.float32)
    inv_pred_emb = np.asarray(inv_pred_emb, dtype=np.float32)
    signs = np.asarray(signs, dtype=np.float32)
    W1 = np.asarray(W1, dtype=np.float32)
    b1 = np.asarray(b1, dtype=np.float32)
    W2 = np.asarray(W2, dtype=np.float32)
    b2 = np.asarray(b2, dtype=np.float32)
    heads = np.asarray(heads).astype(np.int64)
    tails = np.asarray(tails).astype(np.int64)

    msgs = _messages(heads, tails, signs)
    bias_zero = (not b1.any(), not b2.any())
    nc = _build(tuple(msgs), bias_zero=bias_zero)
    in_maps = _prep_inputs(term_emb, pred_emb, inv_pred_emb, W1, b1, W2, b2,
                           msgs)
    res = run_bass_kernel_spmd(nc, in_maps, list(range(N_CORES)))

    out = np.empty((T, B, D), np.float32)
    for c in range(N_CORES):
        o = res.results[c]["outT"].astype(np.float32)
        out[:, c * BC:(c + 1) * BC, :] = o.reshape(D, T, BC).transpose(1, 2, 0)
    return out
